# revision 17
# baseline (speedup 1.0000x reference)
"""CapsNet (conv1 -> caps conv -> squash -> per-position linear -> dynamic
routing) on 8 Trainium2 NeuronCores.

Strategy
--------
The 21.2MB caps-conv weight dominates memory traffic, so that conv is sharded
by output channel (32 ch / core = 2.65MB / core).  Each core then owns 4
complete capsule types (32 channels = 4 types x 8 dims), computes squash and
the per-position 8->16 linear locally, and contributes its 144x16 slice of
uhat (plus a pre-transposed copy) to a single small AllGather (~37KB total).
The sequential 10-iteration routing loop is replicated on every core; the
output is read from core 0.

Numerics: fp32 throughout; sqrt(x) is computed as exp(0.5*ln(x)) so the whole
kernel uses a single ACT table set (exp/ln/relu/square) -- switching table
sets costs ~2.7us on the scalar engine.

b is accumulated directly in PSUM across routing iterations via matmul
accumulation (start=False), in layout [128 partitions = pos-within-tile,
9 tiles x 10 classes] which matches the (32,6,6,10) row-major output order.
"""

import functools
import numpy as np

N_CORES = 8
ROUTING_ITERS = 10

# fixed problem shapes
_POS = 36          # 6x6 output positions of the caps conv
_NTYPES = 32       # capsule types
_NTILE = 9         # 1152 rows / 128 partitions
_M = 10            # routing classes
_O = 16            # capsule output dim
_D = 8             # capsule input dim
_BLK = 4608        # per-core allgather payload (144*16 * 2 layouts)


def _build_program(n_cores=N_CORES, fake_gather=False, repeat_routing=1,
                   repeat_conv=1, routing_iters=ROUTING_ITERS,
                   stop_after=None):
    """Build + compile the SPMD program.  fake_gather / repeat_* /
    stop_after are only used by the development timing harness; the graded
    path uses defaults."""
    import concourse.bass as bass
    import concourse.bacc as bacc
    import concourse.mybir as mybir
    import concourse.tile as tile
    import concourse.masks as masks

    F32 = mybir.dt.float32
    AF = mybir.ActivationFunctionType
    ALU = mybir.AluOpType

    nc = bacc.Bacc("TRN2", target_bir_lowering=False, debug=False,
                   num_devices=n_cores)

    xcol = nc.dram_tensor("xcol", [81, 400], F32, kind="ExternalInput")
    w1t = nc.dram_tensor("w1t", [81, 256], F32, kind="ExternalInput")
    b1 = nc.dram_tensor("b1", [128, 2], F32, kind="ExternalInput")
    wc = nc.dram_tensor("wc", [128, 5184], F32, kind="ExternalInput")
    bc = nc.dram_tensor("bc", [1, 32], F32, kind="ExternalInput")
    wl = nc.dram_tensor("wl", [36, 512], F32, kind="ExternalInput")
    wlb = nc.dram_tensor("wlb", [36, 64], F32, kind="ExternalInput")
    norms_o = nc.dram_tensor("norms", [10], F32, kind="ExternalOutput")
    b_o = nc.dram_tensor("b_out", [1152, 10], F32, kind="ExternalOutput")

    ag_out = nc.dram_tensor("ag_out", [N_CORES * _BLK], F32,
                            addr_space="Shared")

    with tile.TileContext(nc) as tc:
        with (
            tc.tile_pool(name="sb", bufs=1) as pool,
            tc.tile_pool(name="wcpool", bufs=2 if repeat_conv > 1 else 1) as wcpool,
            tc.tile_pool(name="ps", bufs=1, space="PSUM") as psp,
            tc.tile_pool(name="dram", bufs=1, space="DRAM") as dram,
        ):
            def _dummy_finish():
                dmy = pool.tile([128, 90], F32, tag="dmy")
                nc.vector.memset(dmy[:], 0.0)
                nmd = pool.tile([10, 1], F32, tag="nmd")
                nc.vector.memset(nmd[:], 0.0)
                nc.sync.dma_start(
                    b_o.ap().rearrange("(t p) m -> p t m", p=128),
                    dmy[:].rearrange("p (t m) -> p t m", m=10))
                nc.sync.dma_start(
                    norms_o.ap().rearrange("(n o) -> n o", o=1), nmd[:])

            # ---------------- conv1 (replicated) ----------------
            xcol_t = pool.tile([81, 400], F32)
            nc.gpsimd.dma_start(xcol_t[:], xcol.ap())
            w1_t = pool.tile([81, 256], F32)
            nc.gpsimd.dma_start(w1_t[:], w1t.ap())
            b1_t = pool.tile([128, 2], F32)
            nc.gpsimd.dma_start(b1_t[:], b1.ap())
            bc_t = pool.tile([1, 32], F32)
            nc.gpsimd.dma_start(bc_t[:], bc.ap())
            wl_t = pool.tile([36, 512], F32)
            nc.gpsimd.dma_start(wl_t[:], wl.ap())
            wlb_t = pool.tile([36, 64], F32)
            nc.gpsimd.dma_start(wlb_t[:], wlb.ap())
            ones1 = pool.tile([1, 36], F32)
            nc.vector.memset(ones1[:], 1.0)
            ident = pool.tile([64, 64], F32)
            masks.make_identity(nc, ident[:])

            h_t = []
            for j in range(2):
                h_ps = psp.tile([128, 400], F32, tag=f"hps{j}")
                nc.tensor.matmul(h_ps[:], w1_t[:, j * 128:(j + 1) * 128],
                                 xcol_t[:], start=True, stop=True)
                ht = pool.tile([128, 400], F32, tag=f"h{j}")
                nc.scalar.activation(ht[:], h_ps[:], AF.Relu,
                                     bias=b1_t[:, j:j + 1])
                h_t.append(ht)
            h3 = [t[:].rearrange("p (y x) -> p y x", x=20) for t in h_t]
            if stop_after == "conv1":
                _dummy_finish()

            if stop_after not in ("conv1",):
                # --------- caps conv (sharded by out-channel) ---------
                # out p[oc_local, pos] accumulated over 162 K-tiles + bias
                # row.  (the im2col window must be the MOVING operand: the
                # stationary side only allows one free dim at BIR level)
                p_ps = psp.tile([32, 36], F32)
                # 16 chunks spread over HWDGE (sync/scalar-issued) and SWDGE
                # (gpsimd-issued) queues so all 16 DMA engines pull weights.
                chunk_cols = [352, 352] + [320] * 14
                chunk_off = [0]
                for ccols in chunk_cols:
                    chunk_off.append(chunk_off[-1] + ccols)
                for rep in range(repeat_conv):
                    wc_tiles = []
                    for chunk, ccols in enumerate(chunk_cols):
                        t = wcpool.tile([128, ccols], F32,
                                        tag=f"wc{chunk % 8}")
                        eng = nc.sync if chunk % 2 == 0 else nc.gpsimd
                        eng.dma_start(
                            t[:], wc.ap()[:, chunk_off[chunk]:
                                          chunk_off[chunk] + ccols])
                        wc_tiles.append(t)
                    idx = 0
                    for dy in range(9):
                        for dx in range(9):
                            for cb in range(2):
                                col = idx * 32
                                chunk = 0
                                while col >= chunk_off[chunk + 1]:
                                    chunk += 1
                                col -= chunk_off[chunk]
                                win = h3[cb][:, dy:dy + 11:2, dx:dx + 11:2]
                                nc.tensor.matmul(
                                    p_ps[:],
                                    wc_tiles[chunk][:, col:col + 32],
                                    win, start=(idx == 0), stop=False)
                                idx += 1
                    nc.tensor.matmul(p_ps[:], bc_t[:], ones1[:],
                                     start=False, stop=True)
                p_sb = pool.tile([32, 36], F32)
                nc.vector.tensor_copy(p_sb[:], p_ps[:])
                pT_ps = psp.tile([36, 32], F32)
                nc.tensor.transpose(pT_ps[:], p_sb[:], ident[0:32, 0:32])
                if stop_after == "caps":
                    _dummy_finish()

            if stop_after not in ("conv1", "caps"):
                # --------- squash + uhat (local 4 capsule types) ---------
                pT3 = pT_ps[:].rearrange("p (g d) -> p g d", d=8)
                sq_t = pool.tile([36, 32], F32)
                nc.scalar.activation(sq_t[:], pT_ps[:], AF.Square)
                nm2u = pool.tile([36, 4], F32)
                nc.vector.tensor_reduce(
                    nm2u[:], sq_t[:].rearrange("p (g d) -> p g d", d=8),
                    axis=mybir.AxisListType.X, op=ALU.add)
                lnu = pool.tile([36, 4], F32)
                nc.scalar.activation(lnu[:], nm2u[:], AF.Ln)
                nmu = pool.tile([36, 4], F32)
                nc.scalar.activation(nmu[:], lnu[:], AF.Exp, scale=0.5)
                d1u = pool.tile([36, 4], F32)
                nc.vector.tensor_scalar_add(d1u[:], nm2u[:], 1.0)
                rcu = pool.tile([36, 4], F32)
                nc.vector.reciprocal(rcu[:], d1u[:])
                fu = pool.tile([36, 4], F32)
                nc.vector.tensor_tensor(fu[:], nmu[:], rcu[:], op=ALU.mult)
                usq = pool.tile([36, 32], F32)
                nc.vector.tensor_tensor(
                    usq[:].rearrange("p (g d) -> p g d", d=8), pT3,
                    fu[:].unsqueeze(-1).broadcast_to([36, 4, 8]),
                    op=ALU.mult)
                # uhat = sum_d usq * wl + wlb   -> [36, (g,o)]
                prod = pool.tile([36, 512], F32)
                nc.vector.tensor_tensor(
                    prod[:].rearrange("p (g o d) -> p g o d", o=16, d=8),
                    wl_t[:].rearrange("p (g o d) -> p g o d", o=16, d=8),
                    usq[:].rearrange("p (g d) -> p g d", d=8)
                        .unsqueeze(2).broadcast_to([36, 4, 16, 8]),
                    op=ALU.mult)
                uh0 = pool.tile([36, 64], F32)
                nc.vector.tensor_reduce(
                    uh0[:].rearrange("p (g o) -> p g o", o=16),
                    prod[:].rearrange("p (g o d) -> p g o d", o=16, d=8),
                    axis=mybir.AxisListType.X, op=ALU.add)
                uhat_t = pool.tile([36, 64], F32)
                nc.vector.tensor_tensor(uhat_t[:], uh0[:], wlb_t[:],
                                        op=ALU.add)
                if stop_after == "local":
                    _dummy_finish()

            if stop_after not in ("conv1", "caps", "local"):
                # --------- allgather payload (both layouts) ---------
                uhT_ps = psp.tile([64, 36], F32)
                nc.tensor.transpose(uhT_ps[:], uhat_t[:], ident[0:36, 0:36])
                uhT_t = pool.tile([64, 36], F32)
                nc.vector.tensor_copy(uhT_t[:], uhT_ps[:])

                # Per-rank payload layout (f32 offsets), designed so the
                # gathered result loads into SBUF with FOUR big strided
                # DMAs (the routing tile row-order is free: tiles 0..7 are
                # each rank's first 128 local rows, tile 8 collects the 8
                # ranks' 16-row tails):
                #   [   0:2048] uhat rows 0..127   at r*16+o
                #   [2048:4096] uhatT of same rows at 2048+o*128+r
                #   [4096:4352] uhat rows 128..143 at 4096+j*16+o
                #   [4352:4608] uhatT tail         at 4352+o*16+j
                ag_in = dram.tile([_BLK], F32)
                uh3 = uhat_t[:].rearrange("p (g o) -> p g o", o=16)
                nc.sync.dma_start(
                    ag_in[:][0:1728].rearrange("(g p o) -> p g o",
                                               g=3, o=16),
                    uh3[:, 0:3, :])
                nc.scalar.dma_start(
                    ag_in[:][1728:2048].rearrange("(p o) -> p o", o=16),
                    uh3[0:20, 3, :])
                a2 = ag_in[:][2048:4096].rearrange("(o r) -> o r", r=128)
                for g in range(3):
                    eng = nc.sync if g % 2 == 0 else nc.scalar
                    eng.dma_start(a2[:, 36 * g:36 * (g + 1)],
                                  uhT_t[16 * g:16 * (g + 1), :])
                nc.scalar.dma_start(a2[:, 108:128], uhT_t[48:64, 0:20])
                nc.sync.dma_start(
                    ag_in[:][4096:4352].rearrange("(j o) -> j o", o=16),
                    uhat_t[20:36, 48:64])
                nc.scalar.dma_start(
                    ag_in[:][4352:4608].rearrange("(o j) -> o j", j=16),
                    uhT_t[48:64, 20:36])

                if fake_gather:
                    for c in range(N_CORES):
                        eng = nc.sync if c % 2 == 0 else nc.scalar
                        eng.dma_start(
                            ag_out.ap()[c * _BLK:(c + 1) * _BLK], ag_in[:])
                else:
                    nc.gpsimd.collective_compute(
                        "AllGather", mybir.AluOpType.bypass,
                        replica_groups=[list(range(n_cores))],
                        ins=[ag_in[:].opt()], outs=[ag_out.ap().opt()])

                # --------- load U [128, 9*16] and UT [16, 1152] ---------
                U_sb = pool.tile([128, _NTILE * 16], F32)
                UT_sb = pool.tile([16, 1152], F32)
                ag = ag_out.ap()
                agq = ag.rearrange("(c q) -> c q", q=_BLK)
                nc.sync.dma_start(
                    U_sb[:, 0:128].rearrange("p (c o) -> p c o", o=16),
                    ag.rearrange("(c p o) -> p c o", c=N_CORES,
                                 o=16)[0:128])
                nc.scalar.dma_start(
                    U_sb[:, 128:144],
                    agq[:, 4096:4352].rearrange("c (j o) -> c j o", o=16))
                nc.sync.dma_start(
                    UT_sb[:, 0:1024].rearrange("o (c p) -> o c p",
                                               c=N_CORES),
                    agq[:, 2048:4096].rearrange("c (o p) -> o c p",
                                                p=128))
                nc.scalar.dma_start(
                    UT_sb[:, 1024:1152].rearrange("o (c j) -> o c j", j=16),
                    agq[:, 4352:4608].rearrange("c (o j) -> o c j", j=16))
                if stop_after == "gather":
                    _dummy_finish()

            if stop_after is None:
                # ---------------- routing ----------------
                E_t = pool.tile([128, 90], F32)
                C_t = pool.tile([128, 90], F32)
                R0_t = pool.tile([128, 9], F32)
                R_t = pool.tile([128, 9], F32)
                sqS = pool.tile([10, 16], F32)
                nm2 = pool.tile([10, 1], F32)
                lnm = pool.tile([10, 1], F32)
                nmv = pool.tile([10, 1], F32)
                dpl = pool.tile([10, 1], F32)
                rcp = pool.tile([10, 1], F32)
                fsc = pool.tile([10, 1], F32)
                s32 = pool.tile([32, 32], F32)
                st32 = pool.tile([32, 32], F32)
                norms_t = pool.tile([10, 1], F32)
                nc.vector.memset(s32[:], 0.0)

                b_ps = psp.tile([128, 90], F32)
                S_ps = psp.tile([10, 16], F32)

                for rep in range(repeat_routing):
                    for it in range(routing_iters):
                        if it == 0:
                            nc.vector.memset(C_t[:], 0.1)
                        else:
                            nc.scalar.activation(E_t[:], b_ps[:], AF.Exp)
                            nc.vector.tensor_reduce(
                                R0_t[:],
                                E_t[:].rearrange("p (n m) -> p n m", m=10),
                                axis=mybir.AxisListType.X, op=ALU.add)
                            nc.vector.reciprocal(R_t[:], R0_t[:])
                            nc.vector.tensor_tensor(
                                C_t[:].rearrange("p (n m) -> p n m", m=10),
                                E_t[:].rearrange("p (n m) -> p n m", m=10),
                                R_t[:].unsqueeze(-1)
                                    .broadcast_to([128, 9, 10]),
                                op=ALU.mult)
                        for t in range(_NTILE):
                            nc.tensor.matmul(
                                S_ps[:], C_t[:, 10 * t:10 * t + 10],
                                U_sb[:, 16 * t:16 * t + 16],
                                start=(t == 0), stop=(t == _NTILE - 1))
                        nc.scalar.activation(sqS[:], S_ps[:], AF.Square,
                                             accum_out=nm2[:])
                        nc.scalar.activation(lnm[:], nm2[:], AF.Ln)
                        nc.scalar.activation(nmv[:], lnm[:], AF.Exp,
                                             scale=0.5)
                        nc.vector.tensor_scalar_add(dpl[:], nm2[:], 1.0)
                        nc.vector.reciprocal(rcp[:], dpl[:])
                        nc.vector.tensor_tensor(fsc[:], nmv[:], rcp[:],
                                                op=ALU.mult)
                        nc.vector.tensor_scalar_mul(s32[0:10, 0:16],
                                                    S_ps[:], fsc[:])
                        nc.vector.transpose(st32[:], s32[:])
                        for t in range(_NTILE):
                            nc.tensor.matmul(
                                b_ps[:, 10 * t:10 * t + 10],
                                UT_sb[:, 128 * t:128 * (t + 1)],
                                st32[0:16, 0:10],
                                start=(it == 0 and t == 0),
                                stop=(t == _NTILE - 1),
                                skip_group_check=True)
                        if it == routing_iters - 1:
                            nc.vector.tensor_tensor(norms_t[:], nm2[:],
                                                    rcp[:], op=ALU.mult)

                # ---------------- outputs ----------------
                b_sb = pool.tile([128, 90], F32)
                nc.vector.tensor_copy(b_sb[:], b_ps[:])
                # tiles 0..7 hold rank t's local rows 0..127 = global rows
                # 144t+p; tile 8 holds the 8 ranks' 16-row tails.
                nc.sync.dma_start(
                    b_o.ap().rearrange("(t p) m -> p t m",
                                       t=8, p=144)[0:128],
                    b_sb[:, 0:80].rearrange("p (t m) -> p t m", m=10))
                nc.scalar.dma_start(
                    b_o.ap().rearrange("(c z j) m -> z c j m",
                                       c=8, j=16)[8],
                    b_sb[:, 80:90])
                nc.sync.dma_start(
                    norms_o.ap().rearrange("(n o) -> n o", o=1), norms_t[:])

    # Force every ACT instruction onto the one table set that covers all the
    # functions this kernel uses (exp/ln/relu/square/copy).  The default
    # placement pass picks the first set containing each func, which
    # alternates between exp_and_others and natural_log inside the routing
    # loop -- two ~1.3us table loads per iteration.
    _orig_tables = bacc.get_activation_tables
    _keep = "natural_log_exp_and_others"

    def _one_set_tables(arch):
        tabs = _orig_tables(arch)
        return {k: (v if k == _keep else set()) for k, v in tabs.items()}

    bacc.get_activation_tables = _one_set_tables
    try:
        nc.compile()
    finally:
        bacc.get_activation_tables = _orig_tables
    return nc


@functools.lru_cache(maxsize=8)
def _cached_program(n_cores=N_CORES, fake_gather=False, repeat_routing=1,
                    repeat_conv=1, routing_iters=ROUTING_ITERS,
                    stop_after=None):
    return _build_program(n_cores, fake_gather, repeat_routing, repeat_conv,
                          routing_iters, stop_after)


def prep_inputs(x, conv1_w, conv1_b, caps_w, caps_b, W, Wb, n_cores=N_CORES):
    """Host-side layout preparation.  Returns one input map per core."""
    x = np.asarray(x, dtype=np.float32)
    conv1_w = np.asarray(conv1_w, dtype=np.float32)
    conv1_b = np.asarray(conv1_b, dtype=np.float32)
    caps_w = np.asarray(caps_w, dtype=np.float32)
    caps_b = np.asarray(caps_b, dtype=np.float32)
    W = np.asarray(W, dtype=np.float32)
    Wb = np.asarray(Wb, dtype=np.float32)

    img = x[0, 0]                                     # (28, 28)
    sw = np.lib.stride_tricks.sliding_window_view(img, (20, 20))  # (9,9,20,20)
    xcol = np.ascontiguousarray(sw.reshape(81, 400))
    w1t = np.ascontiguousarray(conv1_w.reshape(256, 81).T)
    b1 = np.ascontiguousarray(conv1_b.reshape(2, 128).T)

    in_maps = []
    for c in range(n_cores):
        wsl = caps_w[32 * c:32 * c + 32].reshape(32, 2, 128, 9, 9)
        wch = np.ascontiguousarray(
            np.transpose(wsl, (2, 3, 4, 1, 0)).reshape(128, 5184))
        bch = np.ascontiguousarray(caps_b[32 * c:32 * c + 32].reshape(1, 32))
        wlh = np.ascontiguousarray(
            np.transpose(W[4 * c:4 * c + 4], (1, 2, 0, 3, 4)).reshape(36, 512))
        wlbh = np.ascontiguousarray(
            np.transpose(Wb[4 * c:4 * c + 4], (1, 2, 0, 3)).reshape(36, 64))
        in_maps.append({
            "xcol": xcol, "w1t": w1t, "b1": b1,
            "wc": wch, "bc": bch, "wl": wlh, "wlb": wlbh,
        })
    return in_maps


def kernel(x, conv1_w, conv1_b, caps_w, caps_b, W, Wb):
    from concourse.bass_utils import run_bass_kernel_spmd
    nc = _cached_program()
    in_maps = prep_inputs(x, conv1_w, conv1_b, caps_w, caps_b, W, Wb)
    res = run_bass_kernel_spmd(nc, in_maps, list(range(N_CORES)))
    r0 = res.results[0]
    norms = np.asarray(r0["norms"], dtype=np.float32).reshape(10)
    b = np.asarray(r0["b_out"], dtype=np.float32).reshape(32, 6, 6, 10)
    return norms, b
